# revision 10
# baseline (speedup 1.0000x reference)
"""AdditiveNoise (pink-noise IIR + SNR scaling) on 8 TRN2 NeuronCores.

out = audio + sqrt(mean(audio^2)/100) * pink(white)
pink[0] = 0; pink[i] = 0.02*white[i] + 0.98*pink[i-1]

Strategy:
  * Length dim sharded 8 ways (2^21 elems/core), each core lays its shard
    out as (128 partitions x 16384), partition p owning a contiguous chunk.
  * The IIR is solved with the DVE's native tensor_tensor_scan
    (state = a*state + w). We scan the *raw* recurrence s=0.98*s+w and fold
    the 0.02 and the SNR scale into the final combine (scan is linear).
  * Cross-chunk carries: 0.98^k is negligible in f32 for k >= ~1000, so each
    partition warms up its own state from a 1024-sample halo (the tail of the
    previous chunk, staged host-side). No cross-core carry exchange needed.
  * mean(audio^2): per-core partial via ACT Square+accum, one-partition total
    via ones-matmul, AllGather of 8 scalars, local sum + sqrt, broadcast via
    a second tiny matmul.
  * DVE does only the scans (no fast mode exists for them: ~2 cyc/elem).
    Combines (out = pink*s + audio, in place over pink) are split between
    DVE and GpSimd, which can run concurrently with 1x DVE ops.
  * DMA: audio-weighted interleave on the SP HWDGE FIFO so the global-mean
    chain finishes early while white still streams for the scans; outputs
    go out on the ACT/SP HWDGE rings as combines complete.
"""

import sys

sys.path.insert(0, "/opt/trn_rl_repo")

import numpy as np
import ml_dtypes

import concourse.bacc as bacc
import concourse.mybir as mybir
from concourse.tile import TileContext
from concourse.bass_utils import run_bass_kernel_spmd

L = 16_777_216          # total samples (2^24)
M = 8                   # cores
N = L // M              # 2_097_152 per core
P = 128                 # partitions
C = N // P              # 16384 per-partition chunk
H = 1024                # halo (0.98^1024 ~ 1e-9: carry error invisible in f32)
F = 2048                # free-dim tile
T = C // F              # 8 tiles
A_COEF = 1.0 - 0.02     # 0.98
# s = 0.002*sqrt(sum/L) = sqrt(sum * (0.002^2/L))
S_SCALE = (0.02 * 10.0 ** (-20.0 / 20.0)) ** 2 / L

WHITE_BF16 = True       # ship white/halo as bf16 (halves their DMA traffic)
AD = 2                  # audio DMA chunk = AD consecutive F-tiles (2MB chunks)

F32 = mybir.dt.float32
BF16 = mybir.dt.bfloat16
WDT = BF16 if WHITE_BF16 else F32
AF = mybir.ActivationFunctionType
OP = mybir.AluOpType

_CACHE = {}
LAST_RESULT = None


def _build():
    nc = bacc.Bacc("TRN2", target_bir_lowering=False, debug=False, num_devices=M)
    audio_d = nc.dram_tensor("audio", [P, C], F32, kind="ExternalInput")
    white_d = nc.dram_tensor("white", [P, C], WDT, kind="ExternalInput")
    whalo_d = nc.dram_tensor("whalo", [P, H], WDT, kind="ExternalInput")
    out_d = nc.dram_tensor("out", [P, C], F32, kind="ExternalOutput")

    with TileContext(nc) as tc:
        with (
            tc.tile_pool(name="persist", bufs=1) as persist,
            tc.tile_pool(name="wpool", bufs=3) as wpool,
            tc.tile_pool(name="ppool", bufs=1) as ppool,
            tc.tile_pool(name="psum", bufs=1, space="PSUM") as psum_pool,
            tc.tile_pool(name="dram", bufs=1, space="DRAM") as dram_pool,
        ):
            # -- constants (gpsimd memset: keeps DVE free) --
            acoef = persist.tile([P, F], F32)
            nc.gpsimd.memset(acoef[:], A_COEF)
            ones = persist.tile([P, P], F32)
            nc.gpsimd.memset(ones[:], 1.0)

            audio_sb = persist.tile([P, C], F32)
            nsq = T // AD
            sqacc = persist.tile([P, nsq], F32)
            sqs = persist.tile([P, AD * F], F32)  # scratch for Square's out

            # -- halo first (unblocks the DVE chain), then ALL audio in big
            # chunks (the global-mean chain gates the collective), then white
            wh = wpool.tile([P, H], WDT, tag="wh", bufs=1)
            nc.sync.dma_start(wh[:], whalo_d[:])
            for k in range(nsq):
                lo, hi = k * AD * F, (k + 1) * AD * F
                nc.sync.dma_start(audio_sb[:, lo:hi], audio_d[:, lo:hi])
                nc.scalar.activation(
                    sqs[:], audio_sb[:, lo:hi], AF.Square,
                    accum_out=sqacc[:, k : k + 1],
                )
            wt = {}
            for t in range(T):
                lo, hi = t * F, (t + 1) * F
                w = wpool.tile([P, F], WDT, tag="wt", bufs=5)
                nc.sync.dma_start(w[:], white_d[:, lo:hi])
                wt[t] = w

            # -- global mean(audio^2) --
            part = persist.tile([P, 1], F32)
            tmp_t = persist.tile([P, nsq], F32)
            nc.scalar.activation(tmp_t[:], sqacc[:], AF.Identity, accum_out=part[:])
            tot_ps = psum_pool.tile([P, 1], F32, tag="tot")
            nc.tensor.matmul(tot_ps[:], ones[:], part[:])  # every row = core total
            tot_sb = persist.tile([P, 1], F32)
            nc.scalar.copy(tot_sb[:], tot_ps[:])
            cc_in = dram_pool.tile([P, 1], F32)
            cc_out = dram_pool.tile([P, 1], F32, addr_space="Shared")
            nc.scalar.dma_start(cc_in[:], tot_sb[:])
            nc.gpsimd.collective_compute(
                "AllReduce", OP.add,
                replica_groups=[list(range(M))],
                ins=[cc_in.opt()], outs=[cc_out.opt()],
            )
            gtot = persist.tile([P, 1], F32)
            nc.scalar.dma_start(gtot[:], cc_out[:])
            svec = persist.tile([P, 1], F32)
            nc.scalar.activation(svec[:], gtot[:], AF.Sqrt, scale=float(S_SCALE))

            # -- scans (DVE only), chained along the free dim via `initial` --
            ph = ppool.tile([P, H], F32, tag="ph")
            nc.vector.tensor_tensor_scan(
                ph[:], acoef[:, :H], wh[:], 0.0, OP.mult, OP.add
            )
            pk = []
            prev_last = ph[:, H - 1 : H]
            for t in range(T):
                p = ppool.tile([P, F], F32, tag=f"pk{t}")
                nc.vector.tensor_tensor_scan(
                    p[:], acoef[:], wt[t][:], prev_last, OP.mult, OP.add
                )
                prev_last = p[:, F - 1 : F]
                pk.append(p)

            # -- combines (in place over pk, all on DVE: a 2-src DVE op locks
            # the shared SBUF read port, so GpSimd can't actually help) --
            for t in range(T):
                lo, hi = t * F, (t + 1) * F
                nc.vector.scalar_tensor_tensor(
                    pk[t][:], pk[t][:], svec[:], audio_sb[:, lo:hi],
                    OP.mult, OP.add,
                )
                dma = nc.scalar if t % 2 == 0 else nc.sync
                dma.dma_start(out_d[:, lo:hi], pk[t][:])

    nc.compile()
    return nc


def _shard_inputs(audio, white):
    audio = np.ascontiguousarray(audio, dtype=np.float32)
    white = np.ascontiguousarray(white, dtype=np.float32)
    chunks = white.reshape(L // C, C)  # row r = samples [r*C, (r+1)*C)
    halos = np.concatenate(
        [np.zeros((1, H), np.float32), chunks[:-1, C - H :]], axis=0
    )
    wdt = ml_dtypes.bfloat16 if WHITE_BF16 else np.float32
    in_maps = []
    for m in range(M):
        wsh = white[m * N : (m + 1) * N].reshape(P, C)
        if m == 0:
            wsh = wsh.copy()
            wsh[0, 0] = 0.0  # reference forces pink[0] = 0
        in_maps.append(
            {
                "audio": audio[m * N : (m + 1) * N].reshape(P, C),
                "white": np.ascontiguousarray(wsh.astype(wdt)),
                "whalo": np.ascontiguousarray(halos[m * P : (m + 1) * P].astype(wdt)),
            }
        )
    return in_maps


def kernel(audio, white):
    global LAST_RESULT
    if "nc" not in _CACHE:
        _CACHE["nc"] = _build()
    nc = _CACHE["nc"]
    in_maps = _shard_inputs(audio, white)
    res = run_bass_kernel_spmd(nc, in_maps, core_ids=list(range(M)))
    LAST_RESULT = res
    return np.concatenate([r["out"].reshape(-1) for r in res.results])


if __name__ == "__main__":
    rng = np.random.default_rng(0)
    a = rng.standard_normal(L, dtype=np.float32)
    w = rng.standard_normal(L, dtype=np.float32)
    out = kernel(a, w)
    print("out", out.shape, out.dtype, out[:4])
